# revision 12
# baseline (speedup 1.0000x reference)
"""Trainium2 Bass kernel for spherical deep GMM classifier (DGMMC).

Reference computation (B=8192, D=1024, C=128 classes, K=8 comps, N=C*K=1024):
    bw = clip(bandwidths, 1e-3, 100); a = 1/bw
    log_prob[b,n] = -0.5*(D*log(2pi) + D*log(bw[n]) + sq_dist[b,n]/bw[n])
    log_prob += log_softmax(weights.reshape(C,K),1).reshape(N)
    lse1[b,c]  = LSE_k(log_prob[b,c*K+k]) + log_softmax(priors)[c]
    out[b,c]   = lse1[b,c] - LSE_c(lse1[b,c])

Strategy: data-parallel over batch across 8 cores.  All per-component affine
terms are folded into an augmented GEMM assembled on the host (tiny prep):
    xT_aug     = [x.T; xsq_h; xsq_h; xsq_l; 1; 1]            (D+5, B)
    meansT_aug = [means.T * a; ah_h; ah_l; ah_h; c_h; c_l]   (D+5, N)
with ah = -0.5*a and c[n] = -0.5*(D*log(2pi)+D*log(bw)+m_sq*a)+log_w+log_prior,
each large-magnitude augmented row split into hi/lo fp16 pairs so the fp16
GEMM keeps ~22-bit precision on those rank-2 terms.  PSUM of the GEMM then
holds log_prob (incl. prior) directly; the device does the grouped K=8 LSE
and the row LSE over C.
"""

import math

import numpy as np

B, D, C, K = 8192, 1024, 128, 8
N = C * K
NCORES = 8
BLOC = B // NCORES  # rows per core
P = 128
NAUG = 5  # augmented rows (split fp16 rank-2 correction)
AUG = D + NAUG
NFULL = D // P  # full 128-row contraction chunks
LOG_2PI = math.log(2.0 * math.pi)

GEMM_DTYPE = "float16"

_CACHE: dict = {}


def _build_nc(gemm_dtype: str):
    import concourse.bacc as bacc
    import concourse.mybir as mybir
    import concourse.tile as tile

    f32 = mybir.dt.float32
    gdt = getattr(mybir.dt, gemm_dtype)

    nc = bacc.Bacc(None, target_bir_lowering=False)
    xt = nc.dram_tensor("xt", [AUG, BLOC], gdt, kind="ExternalInput")
    mt = nc.dram_tensor("mt", [AUG, N], gdt, kind="ExternalInput")
    out = nc.dram_tensor("out", [BLOC, C], f32, kind="ExternalOutput")

    NB = BLOC // P  # number of 128-row batch tiles per core
    G = N // K  # groups (= classes = 128)

    with tile.TileContext(nc) as tc:
        with (
            tc.tile_pool(name="resident", bufs=1) as resident,
            tc.tile_pool(name="work", bufs=3) as work,
            tc.tile_pool(name="small", bufs=6) as small,
            tc.tile_pool(name="psum", bufs=4, space="PSUM") as psum_pool,
        ):
            xt_sb = resident.tile([P, NFULL + 1, BLOC], gdt)
            mt_sb = resident.tile([P, NFULL + 1, N], gdt)
            # chunk-wise loads so the first matmuls can start early
            for ch in range(NFULL):
                nc.sync.dma_start(xt_sb[:, ch, :], xt[ch * P : (ch + 1) * P, :])
                nc.scalar.dma_start(mt_sb[:, ch, :], mt[ch * P : (ch + 1) * P, :])
            nc.sync.dma_start(xt_sb[0:NAUG, NFULL, :], xt[D : D + NAUG])
            nc.scalar.dma_start(mt_sb[0:NAUG, NFULL, :], mt[D : D + NAUG])

            for bt in range(NB):
                bsl = slice(bt * P, (bt + 1) * P)
                ps = psum_pool.tile([P, N], f32, tag="ps")
                for ch in range(NFULL + 1):
                    kp = P if ch < NFULL else NAUG
                    lhsT = xt_sb[0:kp, ch, bsl]
                    for h in range(N // 512):
                        nc.tensor.matmul(
                            ps[:, h * 512 : (h + 1) * 512],
                            lhsT,
                            mt_sb[0:kp, ch, h * 512 : (h + 1) * 512],
                            start=(ch == 0),
                            stop=(ch == NFULL),
                        )

                # --- grouped LSE over K=8 within each class ---
                pv = ps.rearrange("p (g k) -> p g k", k=K)  # [P, G, K]
                gmax = small.tile([P, G], f32, tag="gmax")
                nc.vector.tensor_reduce(
                    gmax, pv, axis=mybir.AxisListType.X, op=mybir.AluOpType.max
                )
                ei = work.tile([P, N], f32, tag="ei")
                nc.vector.tensor_tensor(
                    ei.rearrange("p (g k) -> p g k", k=K),
                    pv,
                    gmax[:, :, None].to_broadcast((P, G, K)),
                    mybir.AluOpType.subtract,
                )
                nc.scalar.activation(ei, ei, mybir.ActivationFunctionType.Exp)
                # grouped sum via pairwise tree on GpSimd (SBUF-only engine)
                eiv = ei.rearrange("p (g k) -> p g k", k=K)
                t1 = small.tile([P, G, K // 2], f32, tag="t1")
                nc.gpsimd.tensor_tensor(
                    t1, eiv[:, :, 0::2], eiv[:, :, 1::2], mybir.AluOpType.add
                )
                t2 = small.tile([P, G, K // 4], f32, tag="t2")
                nc.gpsimd.tensor_tensor(
                    t2, t1[:, :, 0::2], t1[:, :, 1::2], mybir.AluOpType.add
                )
                gsum = small.tile([P, G], f32, tag="gsum")
                nc.gpsimd.tensor_tensor(
                    gsum, t2[:, :, 0], t2[:, :, 1], mybir.AluOpType.add
                )
                lse1 = work.tile([P, G], f32, tag="lse1")
                nc.scalar.activation(lse1, gsum, mybir.ActivationFunctionType.Ln)
                nc.gpsimd.tensor_add(lse1, lse1, gmax)

                # --- LSE over classes + normalize ---
                nrmax = small.tile([P, 1], f32, tag="nrmax")
                nc.vector.tensor_reduce(
                    nrmax,
                    lse1,
                    axis=mybir.AxisListType.X,
                    op=mybir.AluOpType.max,
                    negate=True,
                )
                e2 = work.tile([P, C], f32, tag="e2")
                s2 = small.tile([P, 1], f32, tag="s2")
                nc.scalar.activation(
                    e2,
                    lse1,
                    mybir.ActivationFunctionType.Exp,
                    bias=nrmax,
                    accum_out=s2,
                )
                lnz = small.tile([P, 1], f32, tag="lnz")
                nc.scalar.activation(lnz, s2, mybir.ActivationFunctionType.Ln)
                denom = small.tile([P, 1], f32, tag="denom")
                nc.gpsimd.tensor_tensor(
                    denom, lnz, nrmax, mybir.AluOpType.subtract
                )  # ln(s2) + rmax
                ot = work.tile([P, C], f32, tag="ot")
                nc.gpsimd.tensor_scalar_sub(ot, lse1, denom)
                nc.sync.dma_start(out[bsl, :], ot)

    # Make Exp and Ln resolve to the single combined table set so the
    # table-load pass doesn't ping-pong two sets every b-tile.  Keys and
    # their order are preserved (act_func_set_id indexes this dict in
    # insertion order and must keep matching act_info.json); we only strip
    # Exp/Ln from every other set so the combined one is the unique choice.
    orig_tables = bacc.get_activation_tables

    def _exp_ln_combined(arch):
        t = orig_tables(arch)
        combined = "natural_log_exp_and_others"
        if combined not in t:
            return t
        strip = {
            mybir.ActivationFunctionType.Exp,
            mybir.ActivationFunctionType.Ln,
        }
        return {
            k: (v if k == combined else (set(v) - strip)) for k, v in t.items()
        }

    bacc.get_activation_tables = _exp_ln_combined
    try:
        nc.compile()
    finally:
        bacc.get_activation_tables = orig_tables
    return nc


def _split16(v):
    hi = v.astype(np.float16).astype(np.float64)
    lo = v - hi
    return hi, lo


def _host_prep(x, means, bandwidths, weights, priors):
    """Build augmented transposed fp16 operands."""
    x = np.asarray(x, dtype=np.float32)
    means = np.asarray(means, dtype=np.float32)

    bw = np.clip(np.asarray(bandwidths, dtype=np.float64), 0.001, 100.0)
    a = 1.0 / bw
    m_sq = np.einsum("nd,nd->n", means.astype(np.float64), means.astype(np.float64))
    w = np.asarray(weights, dtype=np.float64).reshape(C, K)
    log_w = (w - np.log(np.exp(w - w.max(1, keepdims=True)).sum(1, keepdims=True))
             - w.max(1, keepdims=True)).reshape(N)
    pr = np.asarray(priors, dtype=np.float64)
    log_pri = pr - (np.log(np.exp(pr - pr.max()).sum()) + pr.max())
    cvec = (
        -0.5 * (D * LOG_2PI + D * np.log(bw) + m_sq * a)
        + log_w
        + np.repeat(log_pri, K)
    )
    ah = -0.5 * a

    xsq = np.einsum("bd,bd->b", x.astype(np.float64), x.astype(np.float64))
    xsq_h, xsq_l = _split16(xsq)
    ah_h, ah_l = _split16(ah)
    c_h, c_l = _split16(cvec)
    ones = np.ones_like(xsq)

    xt_aug = np.empty((AUG, B), dtype=np.float16)
    xt_aug[0:D] = x.T.astype(np.float16)
    xt_aug[D + 0] = xsq_h
    xt_aug[D + 1] = xsq_h
    xt_aug[D + 2] = xsq_l
    xt_aug[D + 3] = ones
    xt_aug[D + 4] = ones

    mt_aug = np.empty((AUG, N), dtype=np.float16)
    mt_aug[0:D] = (means.T * a).astype(np.float16)
    mt_aug[D + 0] = ah_h
    mt_aug[D + 1] = ah_l
    mt_aug[D + 2] = ah_h
    mt_aug[D + 3] = c_h
    mt_aug[D + 4] = c_l
    return xt_aug, mt_aug


def _run(x, means, bandwidths, weights, priors, trace=False):
    from concourse.bass_utils import run_bass_kernel_spmd

    key = GEMM_DTYPE
    if key not in _CACHE:
        _CACHE[key] = _build_nc(GEMM_DTYPE)
    nc = _CACHE[key]

    xt_aug, mt_aug = _host_prep(x, means, bandwidths, weights, priors)
    in_maps = [
        {
            "xt": np.ascontiguousarray(xt_aug[:, i * BLOC : (i + 1) * BLOC]),
            "mt": mt_aug,
        }
        for i in range(NCORES)
    ]
    res = run_bass_kernel_spmd(nc, in_maps, core_ids=list(range(NCORES)), trace=trace)
    out = np.concatenate([r["out"] for r in res.results], axis=0)
    return out, res


def kernel(x, means, bandwidths, weights, priors):
    out, _ = _run(x, means, bandwidths, weights, priors, trace=False)
    return out


# revision 13
# speedup vs baseline: 1.2348x; 1.2348x over previous
"""Trainium2 Bass kernel for spherical deep GMM classifier (DGMMC).

Reference computation (B=8192, D=1024, C=128 classes, K=8 comps, N=C*K=1024):
    bw = clip(bandwidths, 1e-3, 100); a = 1/bw
    log_prob[b,n] = -0.5*(D*log(2pi) + D*log(bw[n]) + sq_dist[b,n]/bw[n])
    log_prob += log_softmax(weights.reshape(C,K),1).reshape(N)
    lse1[b,c]  = LSE_k(log_prob[b,c*K+k]) + log_softmax(priors)[c]
    out[b,c]   = lse1[b,c] - LSE_c(lse1[b,c])

Strategy: data-parallel over batch across 8 cores.  All per-component affine
terms are folded into an augmented GEMM assembled on the host (tiny prep):
    xT_aug     = [x.T; xsq_h; xsq_h; xsq_l; 1; 1]            (D+5, B)
    meansT_aug = [means.T * a; ah_h; ah_l; ah_h; c_h; c_l]   (D+5, N)
with ah = -0.5*a and c[n] = -0.5*(D*log(2pi)+D*log(bw)+m_sq*a)+log_w+log_prior,
each large-magnitude augmented row split into hi/lo fp16 pairs so the fp16
GEMM keeps ~22-bit precision on those rank-2 terms.  PSUM of the GEMM then
holds log_prob (incl. prior) directly; the device does the grouped K=8 LSE
and the row LSE over C.
"""

import math

import numpy as np

B, D, C, K = 8192, 1024, 128, 8
N = C * K
NCORES = 8
BLOC = B // NCORES  # rows per core
P = 128
NAUG = 5  # augmented rows (split fp16 rank-2 correction)
AUG = D + NAUG
NFULL = D // P  # full 128-row contraction chunks
LOG_2PI = math.log(2.0 * math.pi)

GEMM_DTYPE = "float16"

_CACHE: dict = {}


def _build_nc(gemm_dtype: str):
    import concourse.bacc as bacc
    import concourse.mybir as mybir
    import concourse.tile as tile

    f32 = mybir.dt.float32
    gdt = getattr(mybir.dt, gemm_dtype)

    nc = bacc.Bacc(None, target_bir_lowering=False)
    xt = nc.dram_tensor("xt", [AUG, BLOC], gdt, kind="ExternalInput")
    mt = nc.dram_tensor("mt", [AUG, N], gdt, kind="ExternalInput")
    out = nc.dram_tensor("out", [BLOC, C], f32, kind="ExternalOutput")

    NB = BLOC // P  # number of 128-row batch tiles per core
    G = N // K  # groups (= classes = 128)

    with tile.TileContext(nc) as tc:
        with (
            tc.tile_pool(name="resident", bufs=1) as resident,
            tc.tile_pool(name="work", bufs=3) as work,
            tc.tile_pool(name="small", bufs=6) as small,
            tc.tile_pool(name="psum", bufs=4, space="PSUM") as psum_pool,
        ):
            xt_sb = resident.tile([P, NFULL + 1, BLOC], gdt)
            mt_sb = resident.tile([P, NFULL + 1, N], gdt)
            # chunk-wise loads so the first matmuls can start early
            for ch in range(NFULL):
                nc.sync.dma_start(xt_sb[:, ch, :], xt[ch * P : (ch + 1) * P, :])
                nc.sync.dma_start(mt_sb[:, ch, :], mt[ch * P : (ch + 1) * P, :])
            nc.sync.dma_start(xt_sb[0:NAUG, NFULL, :], xt[D : D + NAUG])
            nc.sync.dma_start(mt_sb[0:NAUG, NFULL, :], mt[D : D + NAUG])

            for bt in range(NB):
                bsl = slice(bt * P, (bt + 1) * P)
                ps = psum_pool.tile([P, N], f32, tag="ps")
                for ch in range(NFULL + 1):
                    kp = P if ch < NFULL else NAUG
                    lhsT = xt_sb[0:kp, ch, bsl]
                    for h in range(N // 512):
                        nc.tensor.matmul(
                            ps[:, h * 512 : (h + 1) * 512],
                            lhsT,
                            mt_sb[0:kp, ch, h * 512 : (h + 1) * 512],
                            start=(ch == 0),
                            stop=(ch == NFULL),
                        )

                # --- grouped LSE over K=8 within each class ---
                pv = ps.rearrange("p (g k) -> p g k", k=K)  # [P, G, K]
                gmax = small.tile([P, G], f32, tag="gmax")
                nc.vector.tensor_reduce(
                    gmax, pv, axis=mybir.AxisListType.X, op=mybir.AluOpType.max
                )
                ei = work.tile([P, N], f32, tag="ei")
                nc.vector.tensor_tensor(
                    ei.rearrange("p (g k) -> p g k", k=K),
                    pv,
                    gmax[:, :, None].to_broadcast((P, G, K)),
                    mybir.AluOpType.subtract,
                )
                nc.scalar.activation(ei, ei, mybir.ActivationFunctionType.Exp)
                # grouped sum via pairwise tree on GpSimd (SBUF-only engine)
                eiv = ei.rearrange("p (g k) -> p g k", k=K)
                t1 = small.tile([P, G, K // 2], f32, tag="t1")
                nc.gpsimd.tensor_tensor(
                    t1, eiv[:, :, 0::2], eiv[:, :, 1::2], mybir.AluOpType.add
                )
                t2 = small.tile([P, G, K // 4], f32, tag="t2")
                nc.gpsimd.tensor_tensor(
                    t2, t1[:, :, 0::2], t1[:, :, 1::2], mybir.AluOpType.add
                )
                gsum = small.tile([P, G], f32, tag="gsum")
                nc.gpsimd.tensor_tensor(
                    gsum, t2[:, :, 0], t2[:, :, 1], mybir.AluOpType.add
                )
                lse1 = work.tile([P, G], f32, tag="lse1")
                nc.scalar.activation(lse1, gsum, mybir.ActivationFunctionType.Ln)
                nc.gpsimd.tensor_add(lse1, lse1, gmax)

                # --- LSE over classes + normalize ---
                nrmax = small.tile([P, 1], f32, tag="nrmax")
                nc.vector.tensor_reduce(
                    nrmax,
                    lse1,
                    axis=mybir.AxisListType.X,
                    op=mybir.AluOpType.max,
                    negate=True,
                )
                e2 = work.tile([P, C], f32, tag="e2")
                s2 = small.tile([P, 1], f32, tag="s2")
                nc.scalar.activation(
                    e2,
                    lse1,
                    mybir.ActivationFunctionType.Exp,
                    bias=nrmax,
                    accum_out=s2,
                )
                lnz = small.tile([P, 1], f32, tag="lnz")
                nc.scalar.activation(lnz, s2, mybir.ActivationFunctionType.Ln)
                denom = small.tile([P, 1], f32, tag="denom")
                nc.gpsimd.tensor_tensor(
                    denom, lnz, nrmax, mybir.AluOpType.subtract
                )  # ln(s2) + rmax
                ot = work.tile([P, C], f32, tag="ot")
                nc.vector.tensor_scalar_sub(ot, lse1, denom)
                nc.sync.dma_start(out[bsl, :], ot)

    # Make Exp and Ln resolve to the single combined table set so the
    # table-load pass doesn't ping-pong two sets every b-tile.  Keys and
    # their order are preserved (act_func_set_id indexes this dict in
    # insertion order and must keep matching act_info.json); we only strip
    # Exp/Ln from every other set so the combined one is the unique choice.
    orig_tables = bacc.get_activation_tables

    def _exp_ln_combined(arch):
        t = orig_tables(arch)
        combined = "natural_log_exp_and_others"
        if combined not in t:
            return t
        strip = {
            mybir.ActivationFunctionType.Exp,
            mybir.ActivationFunctionType.Ln,
        }
        return {
            k: (v if k == combined else (set(v) - strip)) for k, v in t.items()
        }

    bacc.get_activation_tables = _exp_ln_combined
    try:
        nc.compile()
    finally:
        bacc.get_activation_tables = orig_tables
    return nc


def _split16(v):
    hi = v.astype(np.float16).astype(np.float64)
    lo = v - hi
    return hi, lo


def _host_prep(x, means, bandwidths, weights, priors):
    """Build augmented transposed fp16 operands."""
    x = np.asarray(x, dtype=np.float32)
    means = np.asarray(means, dtype=np.float32)

    bw = np.clip(np.asarray(bandwidths, dtype=np.float64), 0.001, 100.0)
    a = 1.0 / bw
    m_sq = np.einsum("nd,nd->n", means.astype(np.float64), means.astype(np.float64))
    w = np.asarray(weights, dtype=np.float64).reshape(C, K)
    log_w = (w - np.log(np.exp(w - w.max(1, keepdims=True)).sum(1, keepdims=True))
             - w.max(1, keepdims=True)).reshape(N)
    pr = np.asarray(priors, dtype=np.float64)
    log_pri = pr - (np.log(np.exp(pr - pr.max()).sum()) + pr.max())
    cvec = (
        -0.5 * (D * LOG_2PI + D * np.log(bw) + m_sq * a)
        + log_w
        + np.repeat(log_pri, K)
    )
    ah = -0.5 * a

    xsq = np.einsum("bd,bd->b", x.astype(np.float64), x.astype(np.float64))
    xsq_h, xsq_l = _split16(xsq)
    ah_h, ah_l = _split16(ah)
    c_h, c_l = _split16(cvec)
    ones = np.ones_like(xsq)

    xt_aug = np.empty((AUG, B), dtype=np.float16)
    xt_aug[0:D] = x.T.astype(np.float16)
    xt_aug[D + 0] = xsq_h
    xt_aug[D + 1] = xsq_h
    xt_aug[D + 2] = xsq_l
    xt_aug[D + 3] = ones
    xt_aug[D + 4] = ones

    mt_aug = np.empty((AUG, N), dtype=np.float16)
    mt_aug[0:D] = (means.T * a).astype(np.float16)
    mt_aug[D + 0] = ah_h
    mt_aug[D + 1] = ah_l
    mt_aug[D + 2] = ah_h
    mt_aug[D + 3] = c_h
    mt_aug[D + 4] = c_l
    return xt_aug, mt_aug


def _run(x, means, bandwidths, weights, priors, trace=False):
    from concourse.bass_utils import run_bass_kernel_spmd

    key = GEMM_DTYPE
    if key not in _CACHE:
        _CACHE[key] = _build_nc(GEMM_DTYPE)
    nc = _CACHE[key]

    xt_aug, mt_aug = _host_prep(x, means, bandwidths, weights, priors)
    in_maps = [
        {
            "xt": np.ascontiguousarray(xt_aug[:, i * BLOC : (i + 1) * BLOC]),
            "mt": mt_aug,
        }
        for i in range(NCORES)
    ]
    res = run_bass_kernel_spmd(nc, in_maps, core_ids=list(range(NCORES)), trace=trace)
    out = np.concatenate([r["out"] for r in res.results], axis=0)
    return out, res


def kernel(x, means, bandwidths, weights, priors):
    out, _ = _run(x, means, bandwidths, weights, priors, trace=False)
    return out


# revision 14
# speedup vs baseline: 1.2539x; 1.0155x over previous
"""Trainium2 Bass kernel for spherical deep GMM classifier (DGMMC).

Reference computation (B=8192, D=1024, C=128 classes, K=8 comps, N=C*K=1024):
    bw = clip(bandwidths, 1e-3, 100); a = 1/bw
    log_prob[b,n] = -0.5*(D*log(2pi) + D*log(bw[n]) + sq_dist[b,n]/bw[n])
    log_prob += log_softmax(weights.reshape(C,K),1).reshape(N)
    lse1[b,c]  = LSE_k(log_prob[b,c*K+k]) + log_softmax(priors)[c]
    out[b,c]   = lse1[b,c] - LSE_c(lse1[b,c])

Strategy: data-parallel over batch across 8 cores.  All per-component affine
terms are folded into an augmented GEMM assembled on the host (tiny prep):
    xT_aug     = [x.T; xsq_h; xsq_h; xsq_l; 1; 1]            (D+5, B)
    meansT_aug = [means.T * a; ah_h; ah_l; ah_h; c_h; c_l]   (D+5, N)
with ah = -0.5*a and c[n] = -0.5*(D*log(2pi)+D*log(bw)+m_sq*a)+log_w+log_prior,
each large-magnitude augmented row split into hi/lo fp16 pairs so the fp16
GEMM keeps ~22-bit precision on those rank-2 terms.  PSUM of the GEMM then
holds log_prob (incl. prior) directly; the device does the grouped K=8 LSE
and the row LSE over C.
"""

import math

import numpy as np

B, D, C, K = 8192, 1024, 128, 8
N = C * K
NCORES = 8
BLOC = B // NCORES  # rows per core
P = 128
NAUG = 5  # augmented rows (split fp16 rank-2 correction)
AUG = D + NAUG
NFULL = D // P  # full 128-row contraction chunks
LOG_2PI = math.log(2.0 * math.pi)

GEMM_DTYPE = "float16"

_CACHE: dict = {}


def _build_nc(gemm_dtype: str):
    import concourse.bacc as bacc
    import concourse.mybir as mybir
    import concourse.tile as tile

    f32 = mybir.dt.float32
    gdt = getattr(mybir.dt, gemm_dtype)

    nc = bacc.Bacc(None, target_bir_lowering=False)
    xt = nc.dram_tensor("xt", [AUG, BLOC], gdt, kind="ExternalInput")
    mt = nc.dram_tensor("mt", [AUG, N], gdt, kind="ExternalInput")
    out = nc.dram_tensor("out", [BLOC, C], f32, kind="ExternalOutput")

    NB = BLOC // P  # number of 128-row batch tiles per core
    G = N // K  # groups (= classes = 128)

    with tile.TileContext(nc) as tc:
        with (
            tc.tile_pool(name="resident", bufs=1) as resident,
            tc.tile_pool(name="work", bufs=3) as work,
            tc.tile_pool(name="small", bufs=6) as small,
            tc.tile_pool(name="psum", bufs=4, space="PSUM") as psum_pool,
        ):
            xt_sb = resident.tile([P, NFULL + 1, BLOC], gdt)
            mt_sb = resident.tile([P, NFULL + 1, N], gdt)
            # chunk-wise loads so the first matmuls can start early
            for ch in range(NFULL):
                nc.sync.dma_start(xt_sb[:, ch, :], xt[ch * P : (ch + 1) * P, :])
                nc.sync.dma_start(mt_sb[:, ch, :], mt[ch * P : (ch + 1) * P, :])
            nc.sync.dma_start(xt_sb[0:NAUG, NFULL, :], xt[D : D + NAUG])
            nc.sync.dma_start(mt_sb[0:NAUG, NFULL, :], mt[D : D + NAUG])

            for bt in range(NB):
                bsl = slice(bt * P, (bt + 1) * P)
                ps = psum_pool.tile([P, N], f32, tag="ps")
                for ch in range(NFULL + 1):
                    kp = P if ch < NFULL else NAUG
                    lhsT = xt_sb[0:kp, ch, bsl]
                    for h in range(N // 512):
                        nc.tensor.matmul(
                            ps[:, h * 512 : (h + 1) * 512],
                            lhsT,
                            mt_sb[0:kp, ch, h * 512 : (h + 1) * 512],
                            start=(ch == 0),
                            stop=(ch == NFULL),
                        )

                # --- grouped LSE over K=8 within each class ---
                pv = ps.rearrange("p (g k) -> p g k", k=K)  # [P, G, K]
                gmax = small.tile([P, G], f32, tag="gmax")
                nc.vector.tensor_reduce(
                    gmax, pv, axis=mybir.AxisListType.X, op=mybir.AluOpType.max
                )
                ei = work.tile([P, N], f32, tag="ei")
                nc.vector.tensor_tensor(
                    ei.rearrange("p (g k) -> p g k", k=K),
                    pv,
                    gmax[:, :, None].to_broadcast((P, G, K)),
                    mybir.AluOpType.subtract,
                )
                nc.scalar.activation(ei, ei, mybir.ActivationFunctionType.Exp)
                # grouped sum via pairwise tree on GpSimd (SBUF-only engine)
                eiv = ei.rearrange("p (g k) -> p g k", k=K)
                t1 = small.tile([P, G, K // 2], f32, tag="t1")
                nc.gpsimd.tensor_tensor(
                    t1, eiv[:, :, 0::2], eiv[:, :, 1::2], mybir.AluOpType.add
                )
                t2 = small.tile([P, G, K // 4], f32, tag="t2")
                nc.gpsimd.tensor_tensor(
                    t2, t1[:, :, 0::2], t1[:, :, 1::2], mybir.AluOpType.add
                )
                gsum = small.tile([P, G], f32, tag="gsum")
                nc.gpsimd.tensor_tensor(
                    gsum, t2[:, :, 0], t2[:, :, 1], mybir.AluOpType.add
                )
                lse1 = work.tile([P, G], f32, tag="lse1")
                nc.scalar.activation(lse1, gsum, mybir.ActivationFunctionType.Ln)
                nc.vector.tensor_add(lse1, lse1, gmax)

                # --- LSE over classes + normalize ---
                # shift = max_c gmax (computed off the critical chain; valid
                # LSE shift since max_c gmax <= max_c lse1 <= max_c gmax+ln K)
                nrmax = small.tile([P, 1], f32, tag="nrmax")
                nc.vector.tensor_reduce(
                    nrmax,
                    gmax,
                    axis=mybir.AxisListType.X,
                    op=mybir.AluOpType.max,
                    negate=True,
                )
                e2 = work.tile([P, C], f32, tag="e2")
                s2 = small.tile([P, 1], f32, tag="s2")
                nc.scalar.activation(
                    e2,
                    lse1,
                    mybir.ActivationFunctionType.Exp,
                    bias=nrmax,
                    accum_out=s2,
                )
                lnz = small.tile([P, 1], f32, tag="lnz")
                nc.scalar.activation(lnz, s2, mybir.ActivationFunctionType.Ln)
                denom_neg = small.tile([P, 1], f32, tag="denom_neg")
                nc.gpsimd.tensor_tensor(
                    denom_neg, nrmax, lnz, mybir.AluOpType.subtract
                )  # -(ln(s2) + rmax)
                ot = work.tile([P, C], f32, tag="ot")
                nc.scalar.activation(
                    ot, lse1, mybir.ActivationFunctionType.Identity, bias=denom_neg
                )
                nc.sync.dma_start(out[bsl, :], ot)

    # Make Exp and Ln resolve to the single combined table set so the
    # table-load pass doesn't ping-pong two sets every b-tile.  Keys and
    # their order are preserved (act_func_set_id indexes this dict in
    # insertion order and must keep matching act_info.json); we only strip
    # Exp/Ln from every other set so the combined one is the unique choice.
    orig_tables = bacc.get_activation_tables

    def _exp_ln_combined(arch):
        t = orig_tables(arch)
        combined = "natural_log_exp_and_others"
        if combined not in t:
            return t
        strip = {
            mybir.ActivationFunctionType.Exp,
            mybir.ActivationFunctionType.Ln,
        }
        return {
            k: (v if k == combined else (set(v) - strip)) for k, v in t.items()
        }

    bacc.get_activation_tables = _exp_ln_combined
    try:
        nc.compile()
    finally:
        bacc.get_activation_tables = orig_tables
    return nc


def _split16(v):
    hi = v.astype(np.float16).astype(np.float64)
    lo = v - hi
    return hi, lo


def _host_prep(x, means, bandwidths, weights, priors):
    """Build augmented transposed fp16 operands."""
    x = np.asarray(x, dtype=np.float32)
    means = np.asarray(means, dtype=np.float32)

    bw = np.clip(np.asarray(bandwidths, dtype=np.float64), 0.001, 100.0)
    a = 1.0 / bw
    m_sq = np.einsum("nd,nd->n", means.astype(np.float64), means.astype(np.float64))
    w = np.asarray(weights, dtype=np.float64).reshape(C, K)
    log_w = (w - np.log(np.exp(w - w.max(1, keepdims=True)).sum(1, keepdims=True))
             - w.max(1, keepdims=True)).reshape(N)
    pr = np.asarray(priors, dtype=np.float64)
    log_pri = pr - (np.log(np.exp(pr - pr.max()).sum()) + pr.max())
    cvec = (
        -0.5 * (D * LOG_2PI + D * np.log(bw) + m_sq * a)
        + log_w
        + np.repeat(log_pri, K)
    )
    ah = -0.5 * a

    xsq = np.einsum("bd,bd->b", x.astype(np.float64), x.astype(np.float64))
    xsq_h, xsq_l = _split16(xsq)
    ah_h, ah_l = _split16(ah)
    c_h, c_l = _split16(cvec)
    ones = np.ones_like(xsq)

    xt_aug = np.empty((AUG, B), dtype=np.float16)
    xt_aug[0:D] = x.T.astype(np.float16)
    xt_aug[D + 0] = xsq_h
    xt_aug[D + 1] = xsq_h
    xt_aug[D + 2] = xsq_l
    xt_aug[D + 3] = ones
    xt_aug[D + 4] = ones

    mt_aug = np.empty((AUG, N), dtype=np.float16)
    mt_aug[0:D] = (means.T * a).astype(np.float16)
    mt_aug[D + 0] = ah_h
    mt_aug[D + 1] = ah_l
    mt_aug[D + 2] = ah_h
    mt_aug[D + 3] = c_h
    mt_aug[D + 4] = c_l
    return xt_aug, mt_aug


def _run(x, means, bandwidths, weights, priors, trace=False):
    from concourse.bass_utils import run_bass_kernel_spmd

    key = GEMM_DTYPE
    if key not in _CACHE:
        _CACHE[key] = _build_nc(GEMM_DTYPE)
    nc = _CACHE[key]

    xt_aug, mt_aug = _host_prep(x, means, bandwidths, weights, priors)
    in_maps = [
        {
            "xt": np.ascontiguousarray(xt_aug[:, i * BLOC : (i + 1) * BLOC]),
            "mt": mt_aug,
        }
        for i in range(NCORES)
    ]
    res = run_bass_kernel_spmd(nc, in_maps, core_ids=list(range(NCORES)), trace=trace)
    out = np.concatenate([r["out"] for r in res.results], axis=0)
    return out, res


def kernel(x, means, bandwidths, weights, priors):
    out, _ = _run(x, means, bandwidths, weights, priors, trace=False)
    return out
